# revision 1
# baseline (speedup 1.0000x reference)
"""Trainium2 Bass kernel for nn_Attention_944892805701.

Dense transformer attention layer: QKV projection + RoPE + causal GQA SDPA +
output projection. B=2, S=2048, DIM=4096, 32 Q heads / 8 KV heads, hd=128.

Sharding (8 cores): 2 (batch) x 4 (head groups). Core (b, g) computes global
Q heads [8g, 8g+8) / KV heads [2g, 2g+2) of batch b and the corresponding
partial output projection y_partial = att_heads @ Wo[:, o_slice]^T. The host
sums the 4 head-group partials per batch (the tensor-parallel "allreduce",
done on host since full outputs are gathered there anyway).

Per-core device program: bf16 matmul operands (full PE rate + FWL weight
loads; fp32r measured ~2 cyc/row on HW so bf16 is 2x faster), fp32 PSUM
accumulation everywhere, fp32 softmax statistics.

  Phase 1 (per 512-wide s-chunk): xT tiles [din, s] via bf16 DMA transpose
    straight from DRAM (host pre-casts x to bf16), project qT/kT in
    [head_dim, s] layout (RoPE fused into the fp32 PSUM drain, bf16 out)
    and vT -> PE-transposed into natural [s, d] bf16 tiles.
  Phase 2 (per q-chunk, per head): scoresT = kT_tile x qT_chunk in
    [k-part, q-free] layout, exp on ScalarE (1/sqrt(hd) folded into the
    activation scale), causality via restricted column ranges plus one
    triangular mask multiply per diagonal block, denominator = fp32 DVE
    accumulation + fp32r ones-matmul partition reduce, PV accumulated in
    PSUM and drained UNNORMALIZED (bf16) to persistent SBUF tiles.
    Denominators go to DRAM ([1,512] rows); after each chunk one batched
    [8,512] reciprocal + partition_broadcast normalizes the chunk's ao
    tiles in place (off the per-head critical path).
  Phase 3: outT[m,:] = sum_o WoT[o-tile, m-tile].T @ att[o-tile, :] from
    SBUF-resident normalized bf16 attention tiles.

Output per core: outT [4096, 2048] f32 = y_partial^T; host transposes+sums.
"""

import math
from contextlib import ExitStack

import numpy as np
import ml_dtypes

import concourse.bass as bass  # noqa: F401
import concourse.tile as tile
from concourse import bacc, mybir
from concourse.bass_utils import run_bass_kernel_spmd

F32 = mybir.dt.float32
F32R = mybir.dt.float32r
BF16 = mybir.dt.bfloat16

N_CORES = 8
DIM = 4096
N_HEADS = 32
N_KV_HEADS = 8
HEAD_DIM = 128
SEQ = 2048

HQ = N_HEADS // 4      # 8 local q heads
HKV = N_KV_HEADS // 4  # 2 local kv heads

SC = 512
P = 128


def _r(ap):
    return ap.bitcast(F32R)


def build_program(seq=SEQ, dim=DIM, hq=HQ, hkv=HKV, debug=False):
    nrep = hq // hkv
    nch = seq // SC
    ndt = dim // P
    nkt_total = seq // P
    dq = hq * HEAD_DIM
    dkv = hkv * HEAD_DIM
    scale = 1.0 / math.sqrt(HEAD_DIM)

    nc = bacc.Bacc("TRN2", target_bir_lowering=False, debug=False,
                   num_devices=N_CORES)

    xTd = nc.dram_tensor("xT", [dim, seq], BF16, kind="ExternalInput").ap()
    wqT = nc.dram_tensor("wqT", [dim, dq], BF16, kind="ExternalInput").ap()
    wkT = nc.dram_tensor("wkT", [dim, dkv], BF16, kind="ExternalInput").ap()
    wvT = nc.dram_tensor("wvT", [dim, dkv], BF16, kind="ExternalInput").ap()
    wot = nc.dram_tensor("wot", [dim // P, dq, P], BF16,
                         kind="ExternalInput").ap()
    cosT = nc.dram_tensor("cosT", [HEAD_DIM, seq], F32,
                          kind="ExternalInput").ap()
    sinT = nc.dram_tensor("sinT", [HEAD_DIM, seq], F32,
                          kind="ExternalInput").ap()
    tri = nc.dram_tensor("tri", [P, P], BF16, kind="ExternalInput").ap()
    iden = nc.dram_tensor("iden", [P, P], BF16, kind="ExternalInput").ap()
    ones_col = nc.dram_tensor("ones_col", [P, 1], F32R,
                              kind="ExternalInput").ap()
    outT = nc.dram_tensor("outT", [dim, seq], F32, kind="ExternalOutput").ap()
    dbg = {}
    if debug:
        for nm in ("dq0", "dk0"):
            dbg[nm] = nc.dram_tensor(nm, [P, seq], BF16,
                                     kind="ExternalOutput").ap()
        dbg["dv0"] = nc.dram_tensor("dv0", [P, HEAD_DIM], BF16,
                                    kind="ExternalOutput").ap()
        dbg["dao0"] = nc.dram_tensor("dao0", [P, SC], BF16,
                                     kind="ExternalOutput").ap()
        dbg["ddn"] = nc.dram_tensor("ddn", [hq, seq], F32,
                                    kind="ExternalOutput").ap()
        dbg["det0"] = nc.dram_tensor("det0", [P, SC], BF16,
                                     kind="ExternalOutput").ap()

    with ExitStack() as ctx:
        tc = ctx.enter_context(tile.TileContext(nc))
        ws = ctx.enter_context(tc.tile_pool(name="ws", bufs=14))    # f32 512
        wsb = ctx.enter_context(tc.tile_pool(name="wsb", bufs=98))  # bf16 512
        big = ctx.enter_context(tc.tile_pool(name="big", bufs=hq + hkv))
        vp = ctx.enter_context(tc.tile_pool(name="vp", bufs=hkv * nkt_total))
        wqp = ctx.enter_context(tc.tile_pool(name="wqp", bufs=5))
        wkvp = ctx.enter_context(tc.tile_pool(name="wkvp", bufs=8))
        wop = ctx.enter_context(tc.tile_pool(name="wop", bufs=3))
        cns = ctx.enter_context(tc.tile_pool(name="cns", bufs=1))
        ps_a = ctx.enter_context(tc.tile_pool(name="ps_a", bufs=2,
                                              space="PSUM"))
        ps_p = ctx.enter_context(tc.tile_pool(name="ps_p", bufs=2,
                                              space="PSUM"))
        dram = ctx.enter_context(tc.tile_pool(name="dram", bufs=1,
                                              space="DRAM"))

        dn_dram = dram.tile([hq, seq], F32, tag="dn")
        dnr_dram = dram.tile([hq, seq], F32, tag="dnr")

        tri_sb = cns.tile([P, P], BF16, tag="tri")
        nc.sync.dma_start(tri_sb[:], tri[:])
        iden_sb = cns.tile([P, P], BF16, tag="iden")
        nc.sync.dma_start(iden_sb[:], iden[:])
        ones_sb = cns.tile([P, 1], F32R, tag="ones")
        nc.sync.dma_start(ones_sb[:], ones_col[:])

        qTr = [big.tile([P, seq], BF16, tag="big", name=f"qTr{i}")
               for i in range(hq)]
        kTr = [big.tile([P, seq], BF16, tag="big", name=f"kTr{i}")
               for i in range(hkv)]
        v_nat = [[vp.tile([P, HEAD_DIM], BF16, tag="v", name=f"v{g}_{t}")
                  for t in range(nkt_total)] for g in range(hkv)]
        # unnormalized attention output tiles, persistent through phase 3
        ao = [[wsb.tile([P, SC], BF16, tag="wsb", name=f"ao{h}_{cc}")
               for cc in range(nch)] for h in range(hq)]

        def rope_drain(dst, psum, cos_c, sin_c):
            """dst(bf16) = psum*cos + rotate_half(psum)*sin."""
            h = HEAD_DIM // 2
            tmp = ws.tile([P, SC], F32, tag="ws")
            nc.vector.tensor_mul(dst, psum, cos_c[:])
            nc.vector.tensor_mul(tmp[0:h, :], psum[h:P, :], sin_c[0:h, :])
            nc.vector.tensor_mul(tmp[h:P, :], psum[0:h, :], sin_c[h:P, :])
            nc.vector.tensor_sub(dst[0:h, :], dst[0:h, :], tmp[0:h, :])
            nc.vector.tensor_add(dst[h:P, :], dst[h:P, :], tmp[h:P, :])

        def emit_loads(c):
            s0 = c * SC
            t = {}
            t["cos"] = ws.tile([P, SC], F32, tag="ws", name=f"cos{c}")
            nc.sync.dma_start(t["cos"][:], cosT[:, s0:s0 + SC])
            t["sin"] = ws.tile([P, SC], F32, tag="ws", name=f"sin{c}")
            nc.sync.dma_start(t["sin"][:], sinT[:, s0:s0 + SC])
            # xT tiles (plain loads from host-transposed x), interleaved with
            # the hb0 W quads they are first consumed with
            t["xT"] = [wsb.tile([P, SC], BF16, tag="wsb", name=f"xT{c}_{i}")
                       for i in range(ndt)]
            t["wq"] = {}
            for dt4 in range(ndt // 4):
                for j in range(4):
                    dt = dt4 * 4 + j
                    nc.sync.dma_start(
                        t["xT"][dt][:],
                        xTd[dt * P:(dt + 1) * P, s0:s0 + SC])
                wq = wqp.tile([P, 4, 2 * HEAD_DIM], BF16, tag="wq",
                              name=f"wq{c}_0_{dt4}")
                nc.sync.dma_start(
                    wq[:], wqT[dt4 * 4 * P:(dt4 + 1) * 4 * P,
                               0:2 * HEAD_DIM
                               ].rearrange("(d p) f -> p d f", p=P))
                t["wq"][(0, dt4)] = wq
            for hb in range(2, hq, 2):
                for dt4 in range(ndt // 4):
                    wq = wqp.tile([P, 4, 2 * HEAD_DIM], BF16, tag="wq",
                                  name=f"wq{c}_{hb}_{dt4}")
                    nc.sync.dma_start(
                        wq[:], wqT[dt4 * 4 * P:(dt4 + 1) * 4 * P,
                                   hb * HEAD_DIM:(hb + 2) * HEAD_DIM
                                   ].rearrange("(d p) f -> p d f", p=P))
                    t["wq"][(hb, dt4)] = wq
            t["wk"] = []
            for dt4 in range(ndt // 4):
                wk = wkvp.tile([P, 4, dkv], BF16, tag="wkv",
                               name=f"wk{c}_{dt4}")
                nc.sync.dma_start(
                    wk[:], wkT[dt4 * 4 * P:(dt4 + 1) * 4 * P, :
                               ].rearrange("(d p) f -> p d f", p=P))
                t["wk"].append(wk)
            t["wv"] = []
            for dt4 in range(ndt // 4):
                wv = wkvp.tile([P, 4, dkv], BF16, tag="wkv",
                               name=f"wv{c}_{dt4}")
                nc.sync.dma_start(
                    wv[:], wvT[dt4 * 4 * P:(dt4 + 1) * 4 * P, :
                               ].rearrange("(d p) f -> p d f", p=P))
                t["wv"].append(wv)
            return t

        def emit_projections(c, t):
            s0 = c * SC
            cos_c, sin_c, xT = t["cos"], t["sin"], t["xT"]
            for hb in range(0, hq, 2):
                pqs = [ps_p.tile([P, SC], F32, tag="p",
                                 name=f"pq{c}_{hb}_{i}") for i in range(2)]
                for dt4 in range(ndt // 4):
                    wq = t["wq"][(hb, dt4)]
                    for j in range(4):
                        dt = dt4 * 4 + j
                        for i in range(2):
                            nc.tensor.matmul(
                                pqs[i][:],
                                wq[:, j, i * HEAD_DIM:(i + 1) * HEAD_DIM],
                                xT[dt][:],
                                start=(dt == 0), stop=(dt == ndt - 1))
                for i in range(2):
                    rope_drain(qTr[hb + i][:, s0:s0 + SC], pqs[i][:],
                               cos_c, sin_c)
            pks = [ps_p.tile([P, SC], F32, tag="p", name=f"pk{c}_{i}")
                   for i in range(hkv)]
            for dt4 in range(ndt // 4):
                wk = t["wk"][dt4]
                for j in range(4):
                    dt = dt4 * 4 + j
                    for g in range(hkv):
                        nc.tensor.matmul(
                            pks[g][:],
                            wk[:, j, g * HEAD_DIM:(g + 1) * HEAD_DIM],
                            xT[dt][:],
                            start=(dt == 0), stop=(dt == ndt - 1))
            for g in range(hkv):
                rope_drain(kTr[g][:, s0:s0 + SC], pks[g][:], cos_c, sin_c)
            pvs = [ps_p.tile([P, SC], F32, tag="p", name=f"pv{c}_{i}")
                   for i in range(hkv)]
            for dt4 in range(ndt // 4):
                wv = t["wv"][dt4]
                for j in range(4):
                    dt = dt4 * 4 + j
                    for g in range(hkv):
                        nc.tensor.matmul(
                            pvs[g][:],
                            wv[:, j, g * HEAD_DIM:(g + 1) * HEAD_DIM],
                            xT[dt][:],
                            start=(dt == 0), stop=(dt == ndt - 1))
            for g in range(hkv):
                vt_sb = wsb.tile([P, SC], BF16, tag="wsb")
                nc.any.tensor_copy(vt_sb[:], pvs[g][:])
                for st in range(SC // P):
                    pt = ps_a.tile([P, P], BF16, tag="o", bufs=3)
                    nc.tensor.transpose(pt[:], vt_sb[:, st * P:(st + 1) * P],
                                        iden_sb[:])
                    nc.any.tensor_copy(v_nat[g][c * (SC // P) + st][:], pt[:])

        def emit_normalize_start(c):
            s0 = c * SC
            dn_c = ws.tile([P, SC], F32, tag="ws", name=f"dnc{c}")
            nc.sync.dma_start(dn_c[0:hq, :], dn_dram[:, s0:s0 + SC])
            rc_c = ws.tile([P, SC], F32, tag="ws", name=f"rcc{c}")
            nc.vector.reciprocal(rc_c[0:hq, :], dn_c[0:hq, :])
            nc.sync.dma_start(dnr_dram[:, s0:s0 + SC], rc_c[0:hq, :])

        def emit_normalize_head(c, h):
            s0 = c * SC
            rrow = ws.tile([P, SC], F32, tag="ws", name=f"rrow{c}_{h}")
            nc.sync.dma_start(rrow[0:1, :], dnr_dram[h:h + 1, s0:s0 + SC])
            rb = ws.tile([P, SC], F32, tag="ws", name=f"rb{c}_{h}")
            nc.gpsimd.partition_broadcast(rb[:], rrow[0:1, :])
            nc.vector.tensor_mul(ao[h][c][:], ao[h][c][:], rb[:])

        def emit_attention(c):
            s0 = c * SC
            nkt = (c + 1) * (SC // P)

            def emit_scores(h, g, kt):
                rr = kt * P - s0
                jlo = max(0, rr)
                pscr = ps_a.tile([P, SC], F32, tag="s", bufs=3,
                                 name=f"pscr{c}_{h}_{kt}")
                nc.tensor.matmul(
                    pscr[:, jlo:SC],
                    kTr[g][:, kt * P:(kt + 1) * P],
                    qTr[h][:, s0 + jlo:s0 + SC],
                    start=True, stop=True)
                return pscr

            def emit_denom(h, acc):
                pd = ps_a.tile([P, SC], F32, tag="s", bufs=3,
                               name=f"pd{c}_{h}")
                nc.tensor.matmul(pd[0:1, :], ones_sb[:], _r(acc[:]),
                                 start=True, stop=True)
                dps = ws.tile([P, SC], F32, tag="ws", name=f"dps{c}_{h}")
                nc.scalar.copy(dps[0:1, :], pd[0:1, :])
                nc.sync.dma_start(dn_dram[h:h + 1, s0:s0 + SC], dps[0:1, :])

            # flat (h, kt) stream with scores emitted 2 ahead across
            # head boundaries; denominators deferred into the next head
            items = [(h, kt) for h in range(hq) for kt in range(nkt)]
            pipe = {}

            def sc_ahead(i):
                h2, kt2 = items[i]
                pipe[i] = emit_scores(h2, h2 // nrep, kt2)

            sc_ahead(0)
            if len(items) > 1:
                sc_ahead(1)
            pending = None
            accs = {}
            pos = {}
            for i, (h, kt) in enumerate(items):
                g = h // nrep
                if kt == 0:
                    if c > 0:
                        if h == 0:
                            emit_normalize_start(c - 1)
                        emit_normalize_head(c - 1, h)
                    accs[h] = ws.tile([P, SC], F32, tag="ws",
                                      name=f"acc{c}_{h}")
                    pos[h] = ps_a.tile([P, SC], F32, tag="o", bufs=3,
                                       name=f"po{c}_{h}")
                acc, po = accs[h], pos[h]
                rr = kt * P - s0
                jlo = max(0, rr)
                if i + 2 < len(items):
                    sc_ahead(i + 2)
                pscr = pipe.pop(i)
                if kt == 2 and pending is not None:
                    emit_denom(*pending)
                    pending = None
                et = wsb.tile([P, SC], BF16, tag="wsb",
                              name=f"et{c}_{h}_{kt}")
                nc.scalar.activation(
                    et[:, jlo:SC], pscr[:, jlo:SC],
                    mybir.ActivationFunctionType.Exp, scale=scale)
                if rr >= 0:
                    nc.vector.tensor_mul(et[:, jlo:jlo + P],
                                         et[:, jlo:jlo + P], tri_sb[:])
                if debug and c == 0 and h == 0 and kt == 0:
                    nc.sync.dma_start(dbg["det0"][:], et[:])
                if kt == 0:
                    nc.vector.tensor_copy(_r(acc[:]), et[:])
                else:
                    nc.vector.tensor_add(_r(acc[:, jlo:SC]),
                                         acc[:, jlo:SC], et[:, jlo:SC])
                nc.tensor.matmul(
                    po[:, jlo:SC],
                    v_nat[g][kt][:],
                    et[:, jlo:SC],
                    start=(kt == 0), stop=(kt == nkt - 1))
                if kt == nkt - 1:
                    if pending is not None:
                        emit_denom(*pending)
                    pending = (h, acc)
                    nc.any.tensor_copy(ao[h][c][:], po[:])
                    del accs[h], pos[h]
            if pending is not None:
                emit_denom(*pending)
            if c == nch - 1:
                emit_normalize_start(c)
                for h in range(hq):
                    emit_normalize_head(c, h)

        tiles = emit_loads(0)
        for c in range(nch):
            emit_projections(c, tiles)
            if c + 1 < nch:
                tiles = emit_loads(c + 1)
            emit_attention(c)

        if debug:
            nc.sync.dma_start(dbg["dq0"][:], qTr[0][:])
            nc.sync.dma_start(dbg["dk0"][:], kTr[0][:])
            nc.sync.dma_start(dbg["dv0"][:], v_nat[0][0][:])
            nc.sync.dma_start(dbg["ddn"][:], dn_dram[:])

        # --- Phase 3: output projection from SBUF-resident att tiles.
        # Two passes (cc 0..2 then cc 3) so the final chunk's normalization
        # chain overlaps the first pass instead of stalling the PE.
        for cc_pass in ([0, 1, 2], [3]) if nch == 4 else ([list(range(nch))]):
            for m in range(dim // P):
                wo = wop.tile([P, hq, P], BF16, tag="wo")
                nc.scalar.dma_start(
                    wo[:], wot[m].rearrange("(o p) f -> p o f", p=P))
                for cc in cc_pass:
                    py = ps_a.tile([P, SC], F32, tag="s", bufs=3)
                    for o in range(hq):
                        nc.tensor.matmul(
                            py[:], wo[:, o, :], ao[o][cc][:],
                            start=(o == 0), stop=(o == hq - 1))
                    yo = ws.tile([P, SC], F32, tag="ws")
                    nc.vector.tensor_copy(yo[:], py[:])
                    nc.scalar.dma_start(
                        outT[m * P:(m + 1) * P, cc * SC:(cc + 1) * SC], yo[:])

    nc.compile()
    return nc


def make_core_inputs(data, Wq, Wk, Wv, Wo, cos, sin):
    """Build in_maps for the 8 cores. Core id = 4*b + g."""
    bf = ml_dtypes.bfloat16

    def cbf(a):
        return np.ascontiguousarray(np.asarray(a).astype(bf))

    c = np.ascontiguousarray
    dq = HQ * HEAD_DIM
    dkv = HKV * HEAD_DIM
    dim = Wq.shape[1]
    tri_m = np.triu(np.ones((P, P), dtype=bf))
    iden = np.eye(P, dtype=bf)
    ones_col = np.ones((P, 1), dtype=np.float32)
    cosT = c(cos.T.astype(np.float32))
    sinT = c(sin.T.astype(np.float32))
    xt_by_batch = [cbf(data[b].T) for b in range(data.shape[0])]
    in_maps = []
    for core in range(N_CORES):
        b, g = divmod(core, 4)
        qs = slice(g * dq, (g + 1) * dq)
        ks = slice(g * dkv, (g + 1) * dkv)
        woT = Wo[:, qs].T                        # [dq, dim]
        wot = cbf(woT.reshape(dq, dim // P, P).transpose(1, 0, 2))
        in_maps.append({
            "xT": xt_by_batch[b],
            "wqT": cbf(Wq[qs, :].T),
            "wkT": cbf(Wk[ks, :].T),
            "wvT": cbf(Wv[ks, :].T),
            "wot": wot,
            "cosT": cosT,
            "sinT": sinT,
            "tri": tri_m,
            "iden": iden,
            "ones_col": ones_col,
        })
    return in_maps


_COMPILED = {}


def _get_program():
    key = (SEQ, DIM, HQ, HKV)
    if key not in _COMPILED:
        _COMPILED[key] = build_program()
    return _COMPILED[key]


def run(inputs, trace=False, tmpdir=None, trace_cores=None):
    nc = _get_program()
    in_maps = make_core_inputs(
        inputs["data"], inputs["Wq"], inputs["Wk"], inputs["Wv"],
        inputs["Wo"], inputs["cos"], inputs["sin"])
    kw = {}
    if trace:
        kw = dict(trace=True, tmpdir=tmpdir, trace_cores=trace_cores)
    res = run_bass_kernel_spmd(nc, in_maps, list(range(N_CORES)), **kw)
    B = inputs["data"].shape[0]
    out = np.zeros((B, SEQ, DIM), dtype=np.float32)
    for core in range(N_CORES):
        b = core // 4
        out[b] += res.results[core]["outT"].T
    return out, res


def kernel(data, Wq, Wk, Wv, Wo, cos, sin, mask):
    assert np.asarray(mask).size == 1, "only causal (numel==1) mask supported"
    inputs = {
        "data": np.asarray(data, dtype=np.float32),
        "Wq": np.asarray(Wq, dtype=np.float32),
        "Wk": np.asarray(Wk, dtype=np.float32),
        "Wv": np.asarray(Wv, dtype=np.float32),
        "Wo": np.asarray(Wo, dtype=np.float32),
        "cos": np.asarray(cos, dtype=np.float32),
        "sin": np.asarray(sin, dtype=np.float32),
    }
    out, _ = run(inputs)
    return out



# revision 3
# speedup vs baseline: 1.2496x; 1.2496x over previous
"""Trainium2 Bass kernel for nn_Attention_944892805701.

Dense transformer attention layer: QKV projection + RoPE + causal GQA SDPA +
output projection. B=2, S=2048, DIM=4096, 32 Q heads / 8 KV heads, hd=128.

Sharding (8 cores): 2 (batch) x 4 (head groups). Core (b, g) computes global
Q heads [8g, 8g+8) / KV heads [2g, 2g+2) of batch b and the corresponding
partial output projection y_partial = att_heads @ Wo[:, o_slice]^T. The host
sums the 4 head-group partials per batch (the tensor-parallel "allreduce",
done on host since full outputs are gathered there anyway).

Per-core device program, tuned for 100% TensorE occupancy (bf16 matmuls,
N=512 moving operand, ~259ns/MM steady state):

  Projections (per 512-wide s-chunk): xT tiles [din, s] (host pre-transposed
    bf16), qT/kT in [head_dim, s] layout. Each PSUM result is drained with a
    single ScalarE copy (frees the PSUM bank in ~0.6us so the next matmul
    group never stalls and the PE HAM stays warm), RoPE applied on VectorE in
    bf16 (2x DVE mode) off the critical path. vT is drained the same way and
    transposed to natural [s, hd] tiles via DMA-xbar transposes (scalar
    queue), not the PE.

  Attention (per q-chunk, per head): scoresT = kT_tile x qT_chunk in
    [k-part, q-free] layout, exp on ScalarE (1/sqrt(hd) folded into the
    activation scale), causality via restricted column ranges plus one
    triangular mask multiply per diagonal block, denominator = bf16 DVE
    accumulation + ones-matmul partition reduce + reciprocal + gpsimd
    partition broadcast, PV accumulated in PSUM and drained UNNORMALIZED
    (bf16) to persistent SBUF tiles, normalized in place per head.

  Interleave: the projection matmuls for chunk c+1 are emitted as small
    "filler" units between attention items of chunk c, and the output
    projection for chunks 0-2 fills attention of chunk 3, so the TensorE
    queue always has independent work while attention items wait on ScalarE
    exp results. Output projection of chunk 3 runs as a dense tail.

Output per core: outT [4096, 2048] bf16 = y_partial^T; host transposes+sums.
"""

import math
from contextlib import ExitStack

import numpy as np
import ml_dtypes

import concourse.bass as bass  # noqa: F401
import concourse.tile as tile
from concourse import bacc, mybir
from concourse.bass_utils import run_bass_kernel_spmd

F32 = mybir.dt.float32
BF16 = mybir.dt.bfloat16

N_CORES = 8
DIM = 4096
N_HEADS = 32
N_KV_HEADS = 8
HEAD_DIM = 128
SEQ = 2048

HQ = N_HEADS // 4      # 8 local q heads
HKV = N_KV_HEADS // 4  # 2 local kv heads

SC = 512
P = 128


def build_program(seq=SEQ, dim=DIM, hq=HQ, hkv=HKV):
    nrep = hq // hkv
    nch = seq // SC
    ndt = dim // P
    nkt_total = seq // P
    dq = hq * HEAD_DIM
    dkv = hkv * HEAD_DIM
    scale = 1.0 / math.sqrt(HEAD_DIM)

    nc = bacc.Bacc("TRN2", target_bir_lowering=False, debug=False,
                   num_devices=N_CORES)

    xTd = nc.dram_tensor("xT", [dim, seq], BF16, kind="ExternalInput").ap()
    wqT = nc.dram_tensor("wqT", [dim, dq], BF16, kind="ExternalInput").ap()
    wkT = nc.dram_tensor("wkT", [dim, dkv], BF16, kind="ExternalInput").ap()
    wvT = nc.dram_tensor("wvT", [dim, dkv], BF16, kind="ExternalInput").ap()
    wot = nc.dram_tensor("wot", [dim // P, dq, P], BF16,
                         kind="ExternalInput").ap()
    cosT = nc.dram_tensor("cosT", [HEAD_DIM, seq], BF16,
                          kind="ExternalInput").ap()
    sinT = nc.dram_tensor("sinT", [HEAD_DIM, seq], BF16,
                          kind="ExternalInput").ap()
    tri = nc.dram_tensor("tri", [P, P], BF16, kind="ExternalInput").ap()
    ones_col = nc.dram_tensor("ones_col", [P, 1], BF16,
                              kind="ExternalInput").ap()
    outT = nc.dram_tensor("outT", [dim, seq], BF16, kind="ExternalOutput").ap()

    with ExitStack() as ctx:
        tc = ctx.enter_context(tile.TileContext(nc))
        ws = ctx.enter_context(tc.tile_pool(name="ws", bufs=6))     # f32 512
        wsb = ctx.enter_context(tc.tile_pool(name="wsb", bufs=98))  # bf16 512
        big = ctx.enter_context(tc.tile_pool(name="big", bufs=hq + hkv))
        vp = ctx.enter_context(tc.tile_pool(name="vp", bufs=hkv * nkt_total))
        wqp = ctx.enter_context(tc.tile_pool(name="wqp", bufs=5))
        wkvp = ctx.enter_context(tc.tile_pool(name="wkvp", bufs=10))
        wop = ctx.enter_context(tc.tile_pool(name="wop", bufs=3))
        cns = ctx.enter_context(tc.tile_pool(name="cns", bufs=1))
        ps_s = ctx.enter_context(tc.tile_pool(name="ps_s", bufs=3,
                                              space="PSUM"))
        ps_o = ctx.enter_context(tc.tile_pool(name="ps_o", bufs=2,
                                              space="PSUM"))
        ps_p = ctx.enter_context(tc.tile_pool(name="ps_p", bufs=3,
                                              space="PSUM"))

        tri_sb = cns.tile([P, P], BF16, tag="tri")
        nc.sync.dma_start(tri_sb[:], tri[:])
        ones_sb = cns.tile([P, 1], BF16, tag="ones")
        nc.sync.dma_start(ones_sb[:], ones_col[:])

        qTr = [big.tile([P, seq], BF16, tag="big", name=f"qTr{i}")
               for i in range(hq)]
        kTr = [big.tile([P, seq], BF16, tag="big", name=f"kTr{i}")
               for i in range(hkv)]
        v_nat = [[vp.tile([P, HEAD_DIM], BF16, tag="v", name=f"v{g}_{t}")
                  for t in range(nkt_total)] for g in range(hkv)]
        # unnormalized attention output tiles, persistent until out-proj
        ao = [[wsb.tile([P, SC], BF16, tag="wsb", name=f"ao{h}_{cc}")
               for cc in range(nch)] for h in range(hq)]

        def emit_loads(c):
            s0 = c * SC
            t = {}
            t["cos"] = wsb.tile([P, SC], BF16, tag="wsb", name=f"cos{c}")
            nc.sync.dma_start(t["cos"][:], cosT[:, s0:s0 + SC])
            t["sin"] = wsb.tile([P, SC], BF16, tag="wsb", name=f"sin{c}")
            nc.sync.dma_start(t["sin"][:], sinT[:, s0:s0 + SC])
            # xT tiles (plain loads from host-transposed x), interleaved with
            # the hb0 W quads they are first consumed with
            t["xT"] = [wsb.tile([P, SC], BF16, tag="wsb", name=f"xT{c}_{i}")
                       for i in range(ndt)]
            t["wq"] = {}
            for dt4 in range(ndt // 4):
                for j in range(4):
                    dt = dt4 * 4 + j
                    nc.sync.dma_start(
                        t["xT"][dt][:],
                        xTd[dt * P:(dt + 1) * P, s0:s0 + SC])
                wq = wqp.tile([P, 4, 2 * HEAD_DIM], BF16, tag="wq",
                              name=f"wq{c}_0_{dt4}")
                nc.sync.dma_start(
                    wq[:], wqT[dt4 * 4 * P:(dt4 + 1) * 4 * P,
                               0:2 * HEAD_DIM
                               ].rearrange("(d p) f -> p d f", p=P))
                t["wq"][(0, dt4)] = wq
            for hb in range(2, hq, 2):
                for dt4 in range(ndt // 4):
                    wq = wqp.tile([P, 4, 2 * HEAD_DIM], BF16, tag="wq",
                                  name=f"wq{c}_{hb}_{dt4}")
                    nc.sync.dma_start(
                        wq[:], wqT[dt4 * 4 * P:(dt4 + 1) * 4 * P,
                                   hb * HEAD_DIM:(hb + 2) * HEAD_DIM
                                   ].rearrange("(d p) f -> p d f", p=P))
                    t["wq"][(hb, dt4)] = wq
            t["wk"] = []
            for dt4 in range(ndt // 4):
                wk = wkvp.tile([P, 4, dkv], BF16, tag="wkv",
                               name=f"wk{c}_{dt4}")
                nc.sync.dma_start(
                    wk[:], wkT[dt4 * 4 * P:(dt4 + 1) * 4 * P, :
                               ].rearrange("(d p) f -> p d f", p=P))
                t["wk"].append(wk)
            t["wv"] = []
            for dt4 in range(ndt // 4):
                wv = wkvp.tile([P, 4, dkv], BF16, tag="wkv",
                               name=f"wv{c}_{dt4}")
                nc.sync.dma_start(
                    wv[:], wvT[dt4 * 4 * P:(dt4 + 1) * 4 * P, :
                               ].rearrange("(d p) f -> p d f", p=P))
                t["wv"].append(wv)
            return t

        def proj_units(c, t):
            """Generator: emits the chunk-c projections in small units,
            yielding between units so attention items can interleave.

            One head per PSUM slot, 32 accumulating matmuls each; with 3
            slots in the pool a drain has ~3 head-groups (~25us) of runway
            before its slot is reallocated, so drain latency never stalls
            the PE."""
            s0 = c * SC
            cos_c, sin_c, xT = t["cos"], t["sin"], t["xT"]
            h2 = HEAD_DIM // 2

            def rope_drain(dst, psum):
                # dst = psum*cos + rotate_half(psum)*sin, bf16 out.
                # Cross-half reads come from PSUM (exempt from the SBUF
                # same-start-partition rule).
                tmp = wsb.tile([P, SC], BF16, tag="wsb")
                nc.vector.tensor_mul(tmp[0:h2, :], psum[h2:P, :],
                                     sin_c[0:h2, :])
                nc.vector.tensor_mul(tmp[h2:P, :], psum[0:h2, :],
                                     sin_c[h2:P, :])
                nc.vector.tensor_mul(dst, psum[:], cos_c[:])
                nc.vector.tensor_sub(dst[0:h2, :], dst[0:h2, :], tmp[0:h2, :])
                nc.vector.tensor_add(dst[h2:P, :], dst[h2:P, :], tmp[h2:P, :])

            def mm_groups(wtiles, col):
                """32 accumulating matmuls into one fresh psum tile, in
                units of 8; returns the psum tile via closure list."""
                ps = ps_p.tile([P, SC], F32, tag="p")
                def gen():
                    for dt4p in range(ndt // 8):
                        for dt4 in (dt4p * 2, dt4p * 2 + 1):
                            w = wtiles[dt4]
                            for j in range(4):
                                dt = dt4 * 4 + j
                                nc.tensor.matmul(
                                    ps[:],
                                    w[:, j, col * HEAD_DIM:
                                      (col + 1) * HEAD_DIM],
                                    xT[dt][:],
                                    start=(dt == 0), stop=(dt == ndt - 1))
                        yield
                return ps, gen

            for hh in range(hq):
                hb, i = (hh // 2) * 2, hh % 2
                wtiles = [t["wq"][(hb, dt4)] for dt4 in range(ndt // 4)]
                pq, gen = mm_groups(wtiles, i)
                for _ in gen():
                    yield
                rope_drain(qTr[hh][:, s0:s0 + SC], pq[:])
                yield
            for g in range(hkv):
                pk, gen = mm_groups(t["wk"], g)
                for _ in gen():
                    yield
                rope_drain(kTr[g][:, s0:s0 + SC], pk[:])
                yield
            for g in range(hkv):
                pv, gen = mm_groups(t["wv"], g)
                for _ in gen():
                    yield
                vt = wsb.tile([P, SC], BF16, tag="wsb")
                nc.vector.tensor_copy(vt[:], pv[:])
                for st in range(SC // P):
                    nc.scalar.dma_start_transpose(
                        v_nat[g][c * (SC // P) + st][:],
                        vt[:, st * P:(st + 1) * P])
                yield

        N_PROJ_UNITS = (hq + 2 * hkv) * (ndt // 8 + 1)

        def outproj_units(cc_list):
            """Generator: output projection outT[m,:] = sum_o WoT.T @ ao for
            the given chunks, in small units."""
            for m in range(dim // P):
                wo = wop.tile([P, hq, P], BF16, tag="wo")
                nc.scalar.dma_start(
                    wo[:], wot[m].rearrange("(o p) f -> p o f", p=P))
                yield
                for cc in cc_list:
                    py = ps_p.tile([P, SC], F32, tag="p")
                    for ob in range(0, hq, 4):
                        for o in range(ob, ob + 4):
                            nc.tensor.matmul(
                                py[:], wo[:, o, :], ao[o][cc][:],
                                start=(o == 0), stop=(o == hq - 1))
                        yield
                    yo = wsb.tile([P, SC], BF16, tag="wsb")
                    nc.vector.tensor_copy(yo[:], py[:])
                    nc.scalar.dma_start(
                        outT[m * P:(m + 1) * P, cc * SC:(cc + 1) * SC],
                        yo[:])
                    yield

        def n_outproj_units(cc_list):
            return (dim // P) * (1 + len(cc_list) * (hq // 4 + 1))

        def emit_denorm(c2, h, acc):
            """Denominator reduce + reciprocal + broadcast + in-place
            normalize of ao[h][c2]."""
            s0 = c2 * SC  # noqa: F841
            pd = ps_s.tile([P, SC], F32, tag="s", name=f"pd{c2}_{h}")
            nc.tensor.matmul(pd[0:1, :], ones_sb[:], acc[:],
                             start=True, stop=True)
            rrow = ws.tile([P, SC], F32, tag="ws", name=f"rr{c2}_{h}")
            nc.vector.reciprocal(rrow[0:1, :], pd[0:1, :])
            rb = ws.tile([P, SC], F32, tag="ws", name=f"rb{c2}_{h}")
            nc.gpsimd.partition_broadcast(rb[:], rrow[0:1, :])
            nc.vector.tensor_mul(ao[h][c2][:], ao[h][c2][:], rb[:])

        def emit_attention(c, filler=None, n_units=0):
            s0 = c * SC
            nkt = (c + 1) * (SC // P)

            def emit_scores(h, g, kt):
                rr = kt * P - s0
                jlo = max(0, rr)
                pscr = ps_s.tile([P, SC], F32, tag="s",
                                 name=f"pscr{c}_{h}_{kt}")
                nc.tensor.matmul(
                    pscr[:, jlo:SC],
                    kTr[g][:, kt * P:(kt + 1) * P],
                    qTr[h][:, s0 + jlo:s0 + SC],
                    start=True, stop=True)
                return pscr

            items = [(h, kt) for h in range(hq) for kt in range(nkt)]
            rate = (n_units / len(items)) if filler is not None else 0.0
            credit = [0.0]
            exhausted = [filler is None]

            def step_filler():
                if exhausted[0]:
                    return
                credit[0] += rate
                while credit[0] >= 1.0:
                    try:
                        next(filler)
                    except StopIteration:
                        exhausted[0] = True
                        return
                    credit[0] -= 1.0

            # flat (h, kt) stream with scores emitted 2 ahead across
            # head boundaries; denominators deferred into the next head
            pipe = {}

            def sc_ahead(i):
                h2, kt2 = items[i]
                pipe[i] = emit_scores(h2, h2 // nrep, kt2)

            sc_ahead(0)
            if len(items) > 1:
                sc_ahead(1)
            pending = None
            accs = {}
            pos = {}
            for i, (h, kt) in enumerate(items):
                step_filler()
                g = h // nrep
                if kt == 0:
                    accs[h] = wsb.tile([P, SC], BF16, tag="wsb",
                                       name=f"acc{c}_{h}")
                    pos[h] = ps_o.tile([P, SC], F32, tag="o",
                                       name=f"po{c}_{h}")
                acc, po = accs[h], pos[h]
                rr = kt * P - s0
                jlo = max(0, rr)
                if i + 2 < len(items):
                    sc_ahead(i + 2)
                pscr = pipe.pop(i)
                if kt == 2 and pending is not None:
                    emit_denorm(*pending)
                    pending = None
                et = wsb.tile([P, SC], BF16, tag="wsb",
                              name=f"et{c}_{h}_{kt}")
                nc.scalar.activation(
                    et[:, jlo:SC], pscr[:, jlo:SC],
                    mybir.ActivationFunctionType.Exp, scale=scale)
                if rr >= 0:
                    nc.vector.tensor_mul(et[:, jlo:jlo + P],
                                         et[:, jlo:jlo + P], tri_sb[:])
                if kt == 0:
                    nc.vector.tensor_copy(acc[:], et[:])
                else:
                    nc.vector.tensor_add(acc[:, jlo:SC],
                                         acc[:, jlo:SC], et[:, jlo:SC])
                nc.tensor.matmul(
                    po[:, jlo:SC],
                    v_nat[g][kt][:],
                    et[:, jlo:SC],
                    start=(kt == 0), stop=(kt == nkt - 1))
                if kt == nkt - 1:
                    if pending is not None:
                        emit_denorm(*pending)
                    pending = (c, h, accs[h])
                    nc.vector.tensor_copy(ao[h][c][:], po[:])
                    del accs[h], pos[h]
            if pending is not None:
                emit_denorm(*pending)
            if filler is not None and not exhausted[0]:
                for _ in filler:
                    pass

        tiles = emit_loads(0)
        # chunk-0 projections run dense (nothing to interleave yet)
        for _ in proj_units(0, tiles):
            pass
        for c in range(nch):
            if c + 1 < nch:
                nt = emit_loads(c + 1)
                filler = proj_units(c + 1, nt)
                n_units = N_PROJ_UNITS
            else:
                filler = outproj_units(list(range(nch - 1)))
                n_units = n_outproj_units(list(range(nch - 1)))
            emit_attention(c, filler, n_units)
        # dense tail: output projection of the last chunk
        for _ in outproj_units([nch - 1]):
            pass

    nc.compile()
    return nc


def make_core_inputs(data, Wq, Wk, Wv, Wo, cos, sin):
    """Build in_maps for the 8 cores. Core id = 4*b + g."""
    bf = ml_dtypes.bfloat16

    def cbf(a):
        return np.ascontiguousarray(np.asarray(a).astype(bf))

    dq = HQ * HEAD_DIM
    dkv = HKV * HEAD_DIM
    dim = Wq.shape[1]
    tri_m = np.triu(np.ones((P, P), dtype=bf))
    ones_col = np.ones((P, 1), dtype=bf)
    cosT = cbf(cos.T)
    sinT = cbf(sin.T)
    xt_by_batch = [cbf(data[b].T) for b in range(data.shape[0])]
    in_maps = []
    for core in range(N_CORES):
        b, g = divmod(core, 4)
        qs = slice(g * dq, (g + 1) * dq)
        ks = slice(g * dkv, (g + 1) * dkv)
        woT = Wo[:, qs].T                        # [dq, dim]
        wot = cbf(woT.reshape(dq, dim // P, P).transpose(1, 0, 2))
        in_maps.append({
            "xT": xt_by_batch[b],
            "wqT": cbf(Wq[qs, :].T),
            "wkT": cbf(Wk[ks, :].T),
            "wvT": cbf(Wv[ks, :].T),
            "wot": wot,
            "cosT": cosT,
            "sinT": sinT,
            "tri": tri_m,
            "ones_col": ones_col,
        })
    return in_maps


_COMPILED = {}


def _get_program():
    key = (SEQ, DIM, HQ, HKV)
    if key not in _COMPILED:
        _COMPILED[key] = build_program()
    return _COMPILED[key]


def run(inputs, trace=False, tmpdir=None, trace_cores=None):
    nc = _get_program()
    in_maps = make_core_inputs(
        inputs["data"], inputs["Wq"], inputs["Wk"], inputs["Wv"],
        inputs["Wo"], inputs["cos"], inputs["sin"])
    kw = {}
    if trace:
        kw = dict(trace=True, tmpdir=tmpdir, trace_cores=trace_cores)
    res = run_bass_kernel_spmd(nc, in_maps, list(range(N_CORES)), **kw)
    B = inputs["data"].shape[0]
    out = np.zeros((B, SEQ, DIM), dtype=np.float32)
    for core in range(N_CORES):
        b = core // 4
        out[b] += res.results[core]["outT"].T.astype(np.float32)
    return out, res


def kernel(data, Wq, Wk, Wv, Wo, cos, sin, mask):
    assert np.asarray(mask).size == 1, "only causal (numel==1) mask supported"
    inputs = {
        "data": np.asarray(data, dtype=np.float32),
        "Wq": np.asarray(Wq, dtype=np.float32),
        "Wk": np.asarray(Wk, dtype=np.float32),
        "Wv": np.asarray(Wv, dtype=np.float32),
        "Wo": np.asarray(Wo, dtype=np.float32),
        "cos": np.asarray(cos, dtype=np.float32),
        "sin": np.asarray(sin, dtype=np.float32),
    }
    out, _ = run(inputs)
    return out
